# revision 31
# baseline (speedup 1.0000x reference)
"""CoupledClustersLossV2 Trainium2 kernel.

Full inputs in, full output out. Internally shards the embeddings
[16384, 2048] f32 across 8 NeuronCores along the class axis (each class
owns 64 contiguous rows = 32 pos + 32 neg), computes per-class losses on
each core, and averages on the host.

Per-core algorithm (shard = [2048, 2048] f32 = 32 classes), measured
~57.5 us one-shot vs a 54.1 us pure-DMA wall (310 GB/s effective per
core with all 8 cores contending for HBM; single-core is 382 GB/s):
  - 16 x 1 MiB HWDGE loads on one ring (mixing in SWDGE or a second HW
    ring measurably hurts HBM efficiency), 6-deep SBUF buffering.
  - Per 128-row tile (= 2 classes), a constant 128x128 matrix W = I - S
    (S averages the 32 positive rows of each 64-row class block) turns
    fp32r TensorEngine matmuls into centered residuals diff = x - anchor
    in PSUM, in two 1024-col chunks.
  - ScalarE Square+accum per chunk -> d2a/d2b [128, 16] column pairs
    (chunking halves the post-last-byte ACT drain).
  - Tail per 8-tile half (first half hidden under remaining DMA):
    d2 = d2a+d2b, PE-transpose [128, 8] -> [8, 130], per-class min of
    the negative d2 (min commutes with sqrt) into cols 128:130, one
    ScalarE sqrt, hinge via DVE sub+max, ScalarE Square+accum ->
    per-class losses [8, 4], one DMA out. (tensor_tensor_reduce crashes
    the device - NRT_EXEC_UNIT_UNRECOVERABLE - do not use.)
"""

import sys

import numpy as np

for _p in ("/opt/trn_rl_repo",):
    if _p not in sys.path:
        sys.path.append(_p)

import concourse.bacc as bacc
import concourse.mybir as mybir
from concourse import tile
from concourse.bass_utils import run_bass_kernel_spmd

N_CORES = 8
D = 2048
S = 32                 # samples per class per polarity
ROWS_PER_CLASS = 2 * S # 64: 32 pos then 32 neg
C_PER_CORE = 32        # classes per core (256 / 8)
ROWS_PER_CORE = C_PER_CORE * ROWS_PER_CLASS  # 2048
TILES = ROWS_PER_CORE // 128                 # 16 tiles of 128 rows (2 classes)
SUPER = 16                                   # DMA super-tiles (1 tile / 1 MiB each)

USE_FP32R = True   # fast fp32 matmul mode (reduced-precision multiply)
MM_DTYPE = "f32r"  # "f32r" | "bf16" (strided truncated-bf16 view of f32 data)
DMA_MODE = "hwdge" # "hwdge" (single HW ring: fastest) | "swdge" | "alt" | "hwdge2"
NEWTON = False     # one Newton refinement step after ScalarE sqrt
                   # (HW sqrt measures ~7e-6 rel err; refinement unneeded)
TRACE = False      # set True from test harness for a profiled run

F32 = mybir.dt.float32
F32R = mybir.dt.float32r
AF = mybir.ActivationFunctionType
ALU = mybir.AluOpType

_CACHE = {}
LAST_RESULTS = None


def _w_matrix() -> np.ndarray:
    """W[k, m] so that (W.T @ x)[m] = x[m] - mean(pos rows of m's class)."""
    w = np.zeros((128, 128), dtype=np.float32)
    for b in range(2):  # two 64-row class blocks per tile
        o = 64 * b
        for m in range(64):
            w[o + m, o + m] += 1.0
            w[o : o + S, o + m] -= 1.0 / S
    return w


def _inline_tensor(nc, data: np.ndarray, name: str, dtype):
    """nc.inline_tensor with an explicit BIR dtype (e.g. float32r over
    float32 bytes — same width, so the embedded .npy payload is valid)."""
    import base64
    import io

    import concourse.bass as bass

    data = np.ascontiguousarray(data)
    assert mybir.dt.size(dtype) == data.dtype.itemsize
    mls = nc._tensor(name, list(data.shape), dtype, kind="Const", type="DRAM")
    buf = io.BytesIO()
    np.save(buf, data, allow_pickle=False)
    mls.file = f"{name}.npy"
    mls.ant_data = base64.standard_b64encode(buf.getvalue()).decode()
    return bass.DRamTensorHandle(name, list(data.shape), dtype)


def _build(
    margin: float,
    loop_n: int | None = None,
    stage: str = "full",
    super_n: int | None = None,
    dma_mode: str | None = None,
    bufs: int | None = None,
    layout: str = "blk",
    tail_lv: int = 5,
):
    import ml_dtypes

    SUPER = super_n if super_n is not None else globals()["SUPER"]
    DMA_MODE = dma_mode if dma_mode is not None else globals()["DMA_MODE"]
    NBUFS = bufs if bufs is not None else 6
    if isinstance(SUPER, (list, tuple)):
        A_LIST = list(SUPER)  # tiles per super-tile, descending for fast drain
        assert sum(A_LIST) == TILES
        SUPER = len(A_LIST)
    else:
        A_LIST = [TILES // SUPER] * SUPER
    A_OFF = [sum(A_LIST[:i]) for i in range(SUPER)]  # first tile of each super

    nc = bacc.Bacc("TRN2", target_bir_lowering=False, debug=False)
    bf16_mm = MM_DTYPE == "bf16"
    in_dt = F32 if bf16_mm else (F32R if USE_FP32R else F32)
    emb = nc.dram_tensor("emb", [ROWS_PER_CORE, D], in_dt, kind="ExternalInput")
    # [8, 4]: row t, col 2h+j -> class 16h + 2t + j (order-free: host means)
    out = nc.dram_tensor("losses", [8, 4], F32, kind="ExternalOutput")

    if bf16_mm:
        w_const = nc.inline_tensor(
            _w_matrix().astype(ml_dtypes.bfloat16), name="wmat"
        )
        w_dt = mybir.dt.bfloat16
    else:
        w_const = _inline_tensor(nc, _w_matrix(), "wmat", in_dt)
        w_dt = in_dt
    id_const = nc.inline_tensor(np.eye(128, dtype=np.float32), name="ident")

    with tile.TileContext(nc) as tc:
        with (
            tc.tile_pool(name="consts", bufs=1) as cpool,
            tc.tile_pool(name="stats", bufs=1) as spool,
            tc.tile_pool(name="inp", bufs=NBUFS) as ipool,
        ):
            # consts via SWDGE so the SP HWDGE ring starts with bulk data
            w_sb = cpool.tile([128, 128], w_dt)
            nc.gpsimd.dma_start(out=w_sb[:], in_=w_const[:, :])
            id_sb = cpool.tile([128, 128], F32)
            nc.gpsimd.dma_start(out=id_sb[:], in_=id_const[:, :])
            # per-tile squared-distance accumulators: column t of d2a/d2b
            # holds the col-chunk partial sums for rows [128t, 128t+128)
            d2a = spool.tile([128, TILES], F32)
            d2b = spool.tile([128, TILES], F32)
            d2s = spool.tile([128, TILES], F32)

            def _half(h, ptail, tpool, losses):
                """Tail for tiles [8h, 8h+8) = classes [16h, 16h+16):
                sum the chunk accumulators, transpose to t-major [8, 128]
                (t = tile row, free = the 128 rows = [pos32 neg32] x 2
                classes), pre-sqrt per-class min of the negative d2 into
                cols 128:130 (min commutes with sqrt), then sqrt + hinge."""
                sl = slice(8 * h, 8 * h + 8)
                nc.vector.tensor_tensor(
                    d2s[:, sl], d2a[:, sl], d2b[:, sl], op=ALU.add
                )
                d2t = ptail.tile([8, 130], F32, tag=f"d2t{h}")
                nc.tensor.transpose(d2t[:, 0:128], d2s[:, sl], id_sb[:])
                if tail_lv < 2:
                    nc.vector.tensor_copy(losses[:, 2 * h : 2 * h + 2], d2t[:, 0:2])
                    return
                for j in range(2):
                    nc.vector.tensor_reduce(
                        d2t[:, 128 + j : 129 + j],
                        d2t[:, 64 * j + S : 64 * j + 2 * S],
                        axis=mybir.AxisListType.X,
                        op=ALU.min,
                    )
                if tail_lv < 3:
                    nc.vector.tensor_copy(losses[:, 2 * h : 2 * h + 2], d2t[:, 128:130])
                    return
                dist = tpool.tile([8, 130], F32, tag=f"dist{h}")
                nc.scalar.activation(dist[:], d2t[:], AF.Sqrt)
                anm = tpool.tile([8, 2], F32, tag=f"anm{h}")
                nc.vector.tensor_scalar(
                    anm[:], dist[:, 128:130], float(margin), None, op0=ALU.subtract
                )
                if tail_lv < 4:
                    nc.vector.tensor_copy(losses[:, 2 * h : 2 * h + 2], anm[:])
                    return
                for j in range(2):
                    hinge = tpool.tile([8, S], F32, tag=f"hinge{h}{j}")
                    nc.vector.tensor_scalar(
                        hinge[:], dist[:, 64 * j : 64 * j + S], anm[:, j : j + 1],
                        0.0, op0=ALU.subtract, op1=ALU.max,
                    )
                    if tail_lv < 5:
                        nc.vector.tensor_copy(
                            losses[:, 2 * h + j : 2 * h + j + 1], hinge[:, 0:1]
                        )
                        continue
                    hsq = tpool.tile([8, S], F32, tag=f"hsq{h}{j}")
                    nc.scalar.activation(
                        hsq[:], hinge[:], AF.Square,
                        accum_out=losses[:, 2 * h + j : 2 * h + j + 1],
                    )

            def body(_iv=None):
                with (
                    tc.tile_pool(name="pchunk", bufs=3, space="PSUM") as pchunk,
                    tc.tile_pool(name="ptail", bufs=1, space="PSUM") as ptail,
                    tc.tile_pool(name="tail", bufs=1) as tpool,
                ):
                    losses = tpool.tile([8, 4], F32)
                    if stage == "tail":
                        nc.vector.tensor_copy(d2a[:], id_sb[:, 0:TILES])
                        nc.vector.tensor_copy(d2b[:], id_sb[:, 0:TILES])
                        _half(0, ptail, tpool, losses)
                        _half(1, ptail, tpool, losses)
                        nc.sync.dma_start(out=out[:, :], in_=losses[:])
                        return
                    for s_ in range(SUPER):
                        A = A_LIST[s_]
                        # rows [128*off, 128*(off+A)) as [128 parts, A, D]
                        r0 = 128 * A_OFF[s_]
                        if layout == "ilv":
                            sv = emb[r0 : r0 + 128 * A, :].rearrange(
                                "(p a) d -> p a d", p=128
                            )
                        else:
                            sv = emb[r0 : r0 + 128 * A, :].rearrange(
                                "(a p) d -> p a d", p=128
                            )
                        x = ipool.tile([128, A * D], in_dt, tag="x")
                        x3 = x[:, :].rearrange("p (a d) -> p a d", a=A)
                        if DMA_MODE == "hwdge":
                            eng = nc.sync
                        elif DMA_MODE == "swdge":
                            eng = nc.gpsimd
                        elif DMA_MODE == "hwdge2":
                            eng = nc.sync if s_ % 2 == 0 else nc.scalar
                        else:
                            eng = nc.sync if s_ % 2 == 0 else nc.gpsimd
                        eng.dma_start(out=x3, in_=sv)
                        if bf16_mm:
                            # truncated-bf16 view: upper 2 bytes of each f32
                            xmm = x[:, :].bitcast(mybir.dt.bfloat16).rearrange(
                                "p (n two) -> p n two", two=2
                            )[:, :, 1]
                        else:
                            xmm = x[:, :]
                        if stage == "dma":
                            # consume a sliver so the load isn't dead
                            nc.vector.tensor_copy(
                                d2a[0:2, s_ : s_ + 1], x[0:2, 0:1].bitcast(F32)
                            )
                            continue
                        for a in range(A):
                            t = A_OFF[s_] + a
                            # two 1024-col chunks per tile: short drain after
                            # the last byte lands (ACT covers 1024, not 2048)
                            for c_, acc in ((0, d2a), (1, d2b)):
                                chk = pchunk.tile([128, 1024], F32, tag="chk")
                                for n in range(2):
                                    col = a * D + 1024 * c_ + 512 * n
                                    nc.tensor.matmul(
                                        chk[:, 512 * n : 512 * (n + 1)],
                                        w_sb[:],
                                        xmm[:, col : col + 512],
                                        start=True,
                                        stop=True,
                                    )
                                if stage in ("mm", "mm2"):
                                    nc.vector.tensor_copy(
                                        d2a[0:2, t : t + 1], chk[0:2, 0:1]
                                    )
                                    continue
                                # chunk d2: accumulate over free axis
                                nc.scalar.activation(
                                    chk[:], chk[:], AF.Square,
                                    accum_out=acc[:, t : t + 1],
                                )
                        if stage == "full" and A_OFF[s_] + A == 8:
                            _half(0, ptail, tpool, losses)
                    if stage == "full":
                        _half(1, ptail, tpool, losses)
                        nc.sync.dma_start(out=out[:, :], in_=losses[:])

            if loop_n is None:
                body()
            else:
                with tc.For_i(0, loop_n, 1):
                    body()

    nc.compile()
    return nc


def _make_runner(nc):
    """Persistent jitted shard_map executor for `nc` across the 8 cores.
    Mirrors bass2jax.run_bass_via_pjrt but caches the compiled callable so
    repeat kernel() calls skip jax retracing / XLA recompile."""
    import jax
    from jax.sharding import Mesh, NamedSharding, PartitionSpec

    from jax.experimental.shard_map import shard_map

    from concourse import bass2jax
    from concourse.bass2jax import _bass_exec_p, partition_id_tensor

    bass2jax.install_neuronx_cc_hook()
    partition_name = nc.partition_id_tensor.name if nc.partition_id_tensor else None
    in_names, out_names, out_avals, zero_outs = [], [], [], []
    for alloc in nc.m.functions[0].allocations:
        if not isinstance(alloc, mybir.MemoryLocationSet):
            continue
        name = alloc.memorylocations[0].name
        if alloc.kind == "ExternalInput":
            if name != partition_name:
                in_names.append(name)
        elif alloc.kind == "ExternalOutput":
            out_names.append(name)
            shape = tuple(alloc.tensor_shape)
            dtype = mybir.dt.np(alloc.dtype)
            out_avals.append(jax.core.ShapedArray(shape, dtype))
            zero_outs.append(np.zeros(shape, dtype))
    assert in_names == ["emb"] and out_names == ["losses"]
    n_outs = len(out_avals)

    def _body(*args):
        operands = list(args)
        if partition_name is not None:
            operands.append(partition_id_tensor())
        outs = _bass_exec_p.bind(
            *operands,
            out_avals=tuple(out_avals),
            in_names=tuple(in_names + out_names + ([partition_name] if partition_name else [])),
            out_names=tuple(out_names),
            lowering_input_output_aliases=(),
            sim_require_finite=True,
            sim_require_nnan=True,
            nc=nc,
        )
        return tuple(outs)

    devices = jax.devices()[:N_CORES]
    mesh = Mesh(np.asarray(devices), ("core",))
    in_specs = (PartitionSpec("core"),) * (1 + n_outs)
    out_specs = (PartitionSpec("core"),) * n_outs
    sharded = jax.jit(
        shard_map(_body, mesh=mesh, in_specs=in_specs, out_specs=out_specs,
                  check_rep=False),
        keep_unused=True,
    )
    sh = NamedSharding(mesh, PartitionSpec("core"))
    zeros_dev = [
        jax.device_put(np.zeros((N_CORES * z.shape[0], *z.shape[1:]), z.dtype), sh)
        for z in zero_outs
    ]

    def run(emb_full: np.ndarray) -> np.ndarray:
        import jax as _jax

        emb_dev = _jax.device_put(emb_full, sh)
        outs = sharded(emb_dev, *zeros_dev)
        return np.asarray(outs[0])  # [8*8, 4] per-class losses

    return run


def kernel(embeddings, target=None, margin=0.3, n_classes=256, n_samples=32, **_):
    emb = np.ascontiguousarray(np.asarray(embeddings, dtype=np.float32))
    assert emb.shape == (16384, 2048), emb.shape
    assert int(n_classes) == 256 and int(n_samples) == 32

    key = (float(margin), USE_FP32R, MM_DTYPE)
    run = _CACHE.get(key)
    if run is None:
        run = _CACHE[key] = _make_runner(_build(float(margin)))

    per_class = run(emb).reshape(-1)  # 256 classes, permuted (mean-safe)
    return np.float32(per_class.mean())

